# revision 30
# baseline (speedup 1.0000x reference)
"""Trainium2 Bass kernel for nn_Lookahead: depthwise 21-tap lookahead conv.

y[t, b, f] = sum_{c=0}^{20} x[t+c, b, f] * weight[f, c], zero-padded past t=S-1.

Strategy (8 NeuronCores, feature-parallel). The axon tunnel moves data at
only ~30 MB/s aggregate (shared across directions and streams), so
end-to-end time is dominated by wire bytes; everything here is organized
around that:

  - x is quantized host-side to biased uint8 with per-(chunk, feature)
    scales (64 MB up instead of 256 MB fp32): u8 = round(x/sx) + 128 via
    a single multiply-add-truncate (no rint/clip passes). The device
    folds the -128 into the uint8->fp16 upcast, and sx folds into the
    conv weights, so no explicit dequantize anywhere.
  - The banded Toeplitz lhsT T_f[k, m] = w'[f, k-m] (0 <= k-m <= 20) is
    built ON DEVICE from the raw (128, 21) per-core weight via 108 small
    partition-offset SBUF DMAs — no 28 MB Toeplitz upload.
  - y is quantized on device to biased uint8 with per-output-row scales
    from an exact absmax reduce (64 MB down instead of 256 MB fp32);
    the host downloads the exact multiplier the device used and inverts
    it, so reciprocal precision cancels. Quantize uses round-half-up via
    a +128.5 bias with a 126/absmax ceiling, safe under both truncating
    and round-to-nearest float->int conversion.
  - Dispatch is module-cached AOT-compiled jax.jit(shard_map(bass_exec))
    via fast_dispatch_compile (no per-call retrace, C++ fast path, and no
    256 MB zero-donation-buffer upload).
  - The sequence is cut into 5 time-chunks (4+4+4+4+3 slots of 108 output
    rows; input includes the 20-row lookahead halo) pipelined through a
    thread pool: chunk k+1's upload and host quantize overlap chunk k's
    execute/download/dequantize. Per-direction sequencers grant exclusive
    in-order transfer turns so same-direction transfers don't multiplex
    (which would make every download finish late together and serialize
    all the dequantizes into a tail). The last chunk uses its own
    exact-size program so no phantom rows cross the wire.
  - Host buffers persist across calls to avoid first-touch page faults
    (~2s/call on this 1-vCPU host).

Measured: ~1.3e-2 scale-relative absmax error (gate: 2e-2), dominated by
the int8 x quantization; deterministic for fixed inputs.
"""

import threading
from concurrent.futures import ThreadPoolExecutor

import numpy as np

_S, _B, _F, _C = 2048, 32, 1024, 20
_NC = 8
_FS = _F // _NC   # 128 features per core
_ST = 108         # output rows per slot (128 - C)
_QCAP = 126.0     # quant ceiling; margin below 127 keeps +128.5-biased
                  # uint8 in [2, 254.5] under any rounding mode

# chunks: (base output row, slots); 4+4+4+4+3 slots cover 19*108 >= 2048
_CHUNKS = [(0, 4), (432, 4), (864, 4), (1296, 4), (1728, 3)]


def _sin(nsl):   # input rows incl. 20-row halo (last slot reads 128 rows)
    return (nsl - 1) * _ST + 128


def _sout(nsl):
    return nsl * _ST


# extra rows carrying sideband data, so each chunk is ONE upload RPC and
# ONE download RPC (small separate wf/ys transfers cost pure latency):
#   xs gains _WROWS rows holding the per-core (128, 21) fp16 folded weights
#   y gains 1 row whose first 512 B hold the per-row fp32 quant multipliers
_WROWS = 2  # 2 rows * 4096 B >= 128*21*2 B of fp16 weights


_ctx = None          # {nsl: (compiled, in_names, out_names, sharding)}, bufs
_ctx_lock = threading.Lock()
LAST_RESULTS = None  # kept for test harness compat (always None here)

_NET = ThreadPoolExecutor(max_workers=len(_CHUNKS))


class _Sequencer:
    """Grant turns in chunk order. Concurrent same-direction transfers
    multiplex the ~30 MB/s tunnel so they all finish late together;
    exclusive in-order turns let chunk k's dequantize overlap chunk k+1's
    download instead of bunching all dequantizes into a serial tail."""

    def __init__(self):
        self._cv = threading.Condition()
        self._turn = 0

    def wait(self, k):
        with self._cv:
            while self._turn != k:
                self._cv.wait()

    def advance(self):
        with self._cv:
            self._turn += 1
            self._cv.notify_all()

    def reset(self):
        with self._cv:
            self._turn = 0


def _build(nsl):
    import concourse.tile as tile
    from concourse import bacc, mybir

    SIN, SOUT = _sin(nsl), _sout(nsl)
    nc = bacc.Bacc("TRN2", target_bir_lowering=False, debug=False, num_devices=_NC)
    x_d = nc.dram_tensor(
        "xs", [SIN + _WROWS, _B, _FS], mybir.dt.uint8, kind="ExternalInput"
    ).ap()
    y_d = nc.dram_tensor(
        "y", [SOUT + 1, _B, _FS], mybir.dt.uint8, kind="ExternalOutput"
    ).ap()
    # fp16 weight view over the xs sideband rows: bytes are w'[f, c] C-order
    w_d = (
        x_d[SIN : SIN + _WROWS, :, :]
        .rearrange("t b f -> (t b f)")
        .bitcast(mybir.dt.float16)[0 : _FS * (_C + 1)]
        .rearrange("(f c) -> f c", f=_FS, c=_C + 1)
    )
    # fp32 scale view over the y sideband row
    s_d = (
        y_d[SOUT : SOUT + 1, :, :]
        .rearrange("t b f -> (t b f)")
        .bitcast(mybir.dt.float32)[0:128]
        .rearrange("(p o) -> p o", o=1)
    )

    FREE = _B * _FS  # 4096 elements per slot per partition

    with tile.TileContext(nc) as tc:
        with (
            tc.tile_pool(name="xp8", bufs=1) as xp8,
            tc.tile_pool(name="xp16", bufs=1) as xp16,
            tc.tile_pool(name="twp", bufs=1) as twp,
            tc.tile_pool(name="stp", bufs=1) as stp,
            tc.tile_pool(name="y8p", bufs=1) as y8p,
            tc.tile_pool(name="scp", bufs=1) as scp,
            tc.tile_pool(name="psp", bufs=6, space="PSUM") as psp,
        ):
            # w transposed to [c, f], then banded Toeplitz lhsT: for each
            # output column m, T[m+c, f, m] = w'[f, c] — a partition-offset
            # copy of the transposed weight placed diagonally.
            wt = twp.tile([32, _FS], mybir.dt.float16, tag="wt")
            nc.sync.dma_start(out=wt[0 : _C + 1, :], in_=w_d.rearrange("f c -> c f"))
            tw = twp.tile([128, _FS * _ST], mybir.dt.float16, tag="tw")
            nc.gpsimd.memset(tw[:, :], 0.0)
            twv = tw[:].rearrange("p (f m) -> p f m", f=_FS, m=_ST)
            for m in range(_ST):
                nc.sync.dma_start(out=twv[m : m + _C + 1, :, m], in_=wt[0 : _C + 1, :])

            sc = scp.tile([128, 1], mybir.dt.float32, tag="sc")
            inv = scp.tile([128, 1], mybir.dt.float32, tag="inv")

            xt8 = xp8.tile([128, nsl * FREE], mybir.dt.uint8, tag="x8")
            for s in range(nsl):
                nc.sync.dma_start(
                    out=xt8[:, s * FREE : (s + 1) * FREE],
                    in_=x_d[s * _ST : s * _ST + 128, :, :].rearrange("t b f -> t (b f)"),
                )
            # x arrives biased: u8 = round(x/sx) + 128; undo the bias in the
            # same instruction that upcasts to fp16.
            xt16 = xp16.tile([128, nsl * FREE], mybir.dt.float16, tag="x16")
            nc.vector.tensor_scalar_add(xt16[:, :], xt8[:, :], -128.0)
            xrv = xt16[:].rearrange("p (s b f) -> p s b f", s=nsl, b=_B, f=_FS)

            st = stp.tile([128, nsl * FREE], mybir.dt.float16, tag="st")
            stv = st[:].rearrange("p (s b f) -> p f s b", s=nsl, b=_B, f=_FS)

            nfree = nsl * _B
            for fp in range(_FS // 2):
                ps = psp.tile([128, 2 * nfree], mybir.dt.float32, tag="ps")
                for fh in range(2):
                    f = 2 * fp + fh
                    nc.tensor.matmul(
                        ps[0:_ST, fh * nfree : (fh + 1) * nfree],
                        twv[:, f, :],
                        xrv[:, :, :, f],
                        start=True,
                        stop=True,
                    )
                pv = ps[:].rearrange("p (f s b) -> p f s b", f=2, s=nsl, b=_B)
                nc.vector.tensor_copy(
                    stv[0:_ST, 2 * fp : 2 * fp + 2, :, :], pv[0:_ST, :, :, :]
                )

            # exact per-partition (= per output row mod ST) absmax, then
            # y8 = y * (QCAP/absmax) + 128.5 stored as uint8; the exact
            # multiplier inv is downloaded so the host can invert it.
            nc.vector.tensor_reduce(
                sc[:, 0:1],
                st[:, :],
                mybir.AxisListType.X,
                mybir.AluOpType.max,
                apply_absolute_value=True,
            )
            nc.vector.reciprocal(inv[:, 0:1], sc[:, 0:1])
            nc.vector.tensor_scalar_mul(inv[:, 0:1], inv[:, 0:1], _QCAP)
            y8 = y8p.tile([128, nsl * FREE], mybir.dt.uint8, tag="y8")
            nc.vector.tensor_scalar(
                y8[:, :],
                st[:, :],
                inv[:, 0:1],
                128.5,
                mybir.AluOpType.mult,
                mybir.AluOpType.add,
            )
            sv = y8[:].rearrange("p (s b f) -> p s b f", s=nsl, b=_B, f=_FS)
            for s in range(nsl):
                nc.scalar.dma_start(
                    out=y_d[s * _ST : (s + 1) * _ST, :, :].rearrange("t b f -> t (b f)"),
                    in_=sv[0:_ST, s, :, :],
                )
            nc.scalar.dma_start(out=s_d, in_=inv[:, :])
    nc.compile()
    return nc


def _make_dispatch(nsl):
    import jax
    from jax.sharding import Mesh, NamedSharding, PartitionSpec

    try:
        from jax.experimental.shard_map import shard_map
    except ImportError:  # newer jax
        from jax import shard_map  # type: ignore

    from concourse import bass2jax, mybir

    nc = _build(nsl)
    bass2jax.install_neuronx_cc_hook()

    partition_name = nc.partition_id_tensor.name if nc.partition_id_tensor else None
    in_names, out_names, out_avals = [], [], []
    for alloc in nc.m.functions[0].allocations:
        if not isinstance(alloc, mybir.MemoryLocationSet):
            continue
        if alloc.kind == "ExternalInput":
            if alloc.memorylocations[0].name != partition_name:
                in_names.append(alloc.memorylocations[0].name)
        elif alloc.kind == "ExternalOutput":
            out_names.append(alloc.memorylocations[0].name)
            out_avals.append(
                jax.core.ShapedArray(tuple(alloc.tensor_shape), mybir.dt.np(alloc.dtype))
            )

    all_in_names = list(in_names)
    if partition_name is not None:
        all_in_names.append(partition_name)

    def _body(*args):
        operands = list(args)
        if partition_name is not None:
            operands.append(bass2jax.partition_id_tensor())
        outs = bass2jax._bass_exec_p.bind(
            *operands,
            out_avals=tuple(out_avals),
            in_names=tuple(all_in_names),
            out_names=tuple(out_names),
            lowering_input_output_aliases=(),
            sim_require_finite=True,
            sim_require_nnan=True,
            nc=nc,
        )
        return tuple(outs)

    devices = jax.devices()[:_NC]
    mesh = Mesh(np.asarray(devices), ("core",))
    sharding = NamedSharding(mesh, PartitionSpec("core"))
    fn = shard_map(
        _body,
        mesh=mesh,
        in_specs=(PartitionSpec("core"),) * len(in_names),
        out_specs=(PartitionSpec("core"),) * len(out_names),
        check_rep=False,
    )
    shape_map = {
        "xs": ((_NC * (_sin(nsl) + _WROWS), _B, _FS), np.uint8),
    }
    example = [
        jax.ShapeDtypeStruct(*shape_map[n], sharding=sharding) for n in in_names
    ]
    compiled = bass2jax.fast_dispatch_compile(
        lambda: jax.jit(fn).lower(*example).compile()
    )
    return compiled, in_names, out_names, sharding


def _get_ctx():
    global _ctx
    with _ctx_lock:
        if _ctx is None:
            progs = {nsl: _make_dispatch(nsl) for nsl in sorted({c[1] for c in _CHUNKS})}
            # persistent staging; biased-zero padding rows stay 128 forever
            xq = [
                np.full((_NC * (_sin(nsl) + _WROWS), _B, _FS), 128, np.uint8)
                for _, nsl in _CHUNKS
            ]
            y = np.zeros((_S, _B, _F), np.float32)  # touch pages once here
            _ctx = (progs, xq, y)
    return _ctx


def _quantize_chunk(x, weight, k, xq_k):
    """Quantize chunk k of x to biased uint8 into the persistent buffer
    xq_k and return the per-core folded fp16 weights.

    u8 = trunc(x/sx + 128.5) = round_half_up(x/sx) + 128, in [1, 255] by
    construction (|x/sx| <= 127), so no rint/clip passes are needed.
    """
    base, nsl = _CHUNKS[k]
    SIN = _sin(nsl)
    STRIDE = SIN + _WROWS
    real = min(SIN, _S - base)
    xs = x[base : base + real]
    amax = np.maximum(xs.max(axis=(0, 1)), -xs.min(axis=(0, 1)))
    amax = np.maximum(amax, 1e-20).astype(np.float32)
    inv_sx = (127.0 / amax).astype(np.float32)
    w_all = (weight * (amax / 127.0)[:, None]).astype(np.float16)  # (F, 21)
    for c in range(_NC):
        fs = c * _FS
        v = xs[:, :, fs : fs + _FS] * inv_sx[fs : fs + _FS]
        v += 128.5
        xq_k[c * STRIDE : c * STRIDE + real] = v.astype(np.uint8)
        # rows past the end stay biased-zero (128) from init; the sideband
        # rows carry this core's folded weights as raw fp16 bytes
        side = xq_k[c * STRIDE + SIN : c * STRIDE + SIN + _WROWS].reshape(-1)
        wb = w_all[fs : fs + _FS].tobytes()
        side[: len(wb)] = np.frombuffer(wb, np.uint8)


def _run_chunk(k, xq_k, progs, y_out, up_seq, down_seq):
    """Upload, execute, download, dequantize one chunk (runs in a pool
    thread; the transfers release the GIL so chunks overlap)."""
    import jax

    base, nsl = _CHUNKS[k]
    SOUT = _sout(nsl)
    OSTRIDE = SOUT + 1
    compiled, in_names, out_names, sharding = progs[nsl]
    up_seq.wait(k)
    try:
        arg = jax.device_put(xq_k, sharding)
        arg.block_until_ready()
    finally:
        up_seq.advance()
    outs = compiled(arg)
    # wait for the execute BEFORE claiming the download turn, so the
    # exclusive down-pipe turn is only held while bytes actually move
    outs[0].block_until_ready()
    down_seq.wait(k)
    try:
        y_u8 = np.asarray(outs[0])  # (NC*(SOUT+1), B, FS) uint8
    finally:
        down_seq.advance()

    rows = min(SOUT, _S - base)
    pp = np.arange(rows) % _ST
    tmp = np.empty((rows, _B, _FS), np.float32)
    for c in range(_NC):
        inv_dl = y_u8[c * OSTRIDE + SOUT].reshape(-1)[:512].view(np.float32)
        s_row = np.float32(1.0) / inv_dl[pp]
        # single-pass cast+unbias, then scale directly into the strided view
        np.subtract(
            y_u8[c * OSTRIDE : c * OSTRIDE + rows], np.float32(128.0),
            out=tmp, dtype=np.float32,
        )
        np.multiply(
            tmp, s_row[:, None, None],
            out=y_out[base : base + rows, :, c * _FS : (c + 1) * _FS],
        )


def kernel(x: np.ndarray, weight: np.ndarray) -> np.ndarray:
    progs, xq_bufs, y_out = _get_ctx()

    x = np.asarray(x)
    weight = np.asarray(weight)

    up_seq, down_seq = _Sequencer(), _Sequencer()
    futs = []
    for k in range(len(_CHUNKS)):
        _quantize_chunk(x, weight, k, xq_bufs[k])
        futs.append(
            _NET.submit(_run_chunk, k, xq_bufs[k], progs, y_out, up_seq, down_seq)
        )
    for f in futs:
        f.result()
    return y_out


# revision 31
# speedup vs baseline: 1.0835x; 1.0835x over previous
"""Trainium2 Bass kernel for nn_Lookahead: depthwise 21-tap lookahead conv.

y[t, b, f] = sum_{c=0}^{20} x[t+c, b, f] * weight[f, c], zero-padded past t=S-1.

Strategy (8 NeuronCores, feature-parallel). The axon tunnel moves data at
only ~30 MB/s aggregate (shared across directions and streams), so
end-to-end time is dominated by wire bytes; everything here is organized
around that:

  - x is quantized host-side to biased uint8 with per-(chunk, feature)
    scales (64 MB up instead of 256 MB fp32): u8 = round(x/sx) + 128 via
    a single multiply-add-truncate (no rint/clip passes). The device
    folds the -128 into the uint8->fp16 upcast, and sx folds into the
    conv weights, so no explicit dequantize anywhere.
  - The banded Toeplitz lhsT T_f[k, m] = w'[f, k-m] (0 <= k-m <= 20) is
    built ON DEVICE from the raw (128, 21) per-core weight via 108 small
    partition-offset SBUF DMAs — no 28 MB Toeplitz upload.
  - y is quantized on device to biased uint8 with per-output-row scales
    from an exact absmax reduce (64 MB down instead of 256 MB fp32);
    the host downloads the exact multiplier the device used and inverts
    it, so reciprocal precision cancels. Quantize uses round-half-up via
    a +128.5 bias with a 126/absmax ceiling, safe under both truncating
    and round-to-nearest float->int conversion.
  - Dispatch is module-cached AOT-compiled jax.jit(shard_map(bass_exec))
    via fast_dispatch_compile (no per-call retrace, C++ fast path, and no
    256 MB zero-donation-buffer upload).
  - The sequence is cut into 5 time-chunks (4+4+4+4+3 slots of 108 output
    rows; input includes the 20-row lookahead halo) pipelined through a
    thread pool: chunk k+1's upload and host quantize overlap chunk k's
    execute/download/dequantize. Per-direction sequencers grant exclusive
    in-order transfer turns so same-direction transfers don't multiplex
    (which would make every download finish late together and serialize
    all the dequantizes into a tail). The last chunk uses its own
    exact-size program so no phantom rows cross the wire.
  - Host buffers persist across calls to avoid first-touch page faults
    (~2s/call on this 1-vCPU host).

Measured: ~1.3e-2 scale-relative absmax error (gate: 2e-2), dominated by
the int8 x quantization; deterministic for fixed inputs.
"""

import threading
from concurrent.futures import ThreadPoolExecutor

import numpy as np

_S, _B, _F, _C = 2048, 32, 1024, 20
_NC = 8
_FS = _F // _NC   # 128 features per core
_ST = 108         # output rows per slot (128 - C)
_QCAP = 126.0     # quant ceiling; margin below 127 keeps +128.5-biased
                  # uint8 in [2, 254.5] under any rounding mode

# chunks: (base output row, slots); 2+5+5+4+3 slots cover 19*108 >= 2048.
# The first chunk is small so the down-pipe starts ~0.3s earlier (its
# quantize+upload is the only serial prefix); the last is small so the
# final download+dequantize tail is short.
_CHUNKS = [(0, 2), (216, 5), (756, 5), (1296, 4), (1728, 3)]


def _sin(nsl):   # input rows incl. 20-row halo (last slot reads 128 rows)
    return (nsl - 1) * _ST + 128


def _sout(nsl):
    return nsl * _ST


# extra rows carrying sideband data, so each chunk is ONE upload RPC and
# ONE download RPC (small separate wf/ys transfers cost pure latency):
#   xs gains _WROWS rows holding the per-core (128, 21) fp16 folded weights
#   y gains 1 row whose first 512 B hold the per-row fp32 quant multipliers
_WROWS = 2  # 2 rows * 4096 B >= 128*21*2 B of fp16 weights


_ctx = None          # {nsl: (compiled, in_names, out_names, sharding)}, bufs
_ctx_lock = threading.Lock()
LAST_RESULTS = None  # kept for test harness compat (always None here)

_NET = ThreadPoolExecutor(max_workers=len(_CHUNKS))


class _Sequencer:
    """Grant turns in chunk order. Concurrent same-direction transfers
    multiplex the ~30 MB/s tunnel so they all finish late together;
    exclusive in-order turns let chunk k's dequantize overlap chunk k+1's
    download instead of bunching all dequantizes into a serial tail."""

    def __init__(self):
        self._cv = threading.Condition()
        self._turn = 0

    def wait(self, k):
        with self._cv:
            while self._turn != k:
                self._cv.wait()

    def advance(self):
        with self._cv:
            self._turn += 1
            self._cv.notify_all()

    def reset(self):
        with self._cv:
            self._turn = 0


def _build(nsl):
    import concourse.tile as tile
    from concourse import bacc, mybir

    SIN, SOUT = _sin(nsl), _sout(nsl)
    nc = bacc.Bacc("TRN2", target_bir_lowering=False, debug=False, num_devices=_NC)
    x_d = nc.dram_tensor(
        "xs", [SIN + _WROWS, _B, _FS], mybir.dt.uint8, kind="ExternalInput"
    ).ap()
    y_d = nc.dram_tensor(
        "y", [SOUT + 1, _B, _FS], mybir.dt.uint8, kind="ExternalOutput"
    ).ap()
    # fp16 weight view over the xs sideband rows: bytes are w'[f, c] C-order
    w_d = (
        x_d[SIN : SIN + _WROWS, :, :]
        .rearrange("t b f -> (t b f)")
        .bitcast(mybir.dt.float16)[0 : _FS * (_C + 1)]
        .rearrange("(f c) -> f c", f=_FS, c=_C + 1)
    )
    # fp32 scale view over the y sideband row
    s_d = (
        y_d[SOUT : SOUT + 1, :, :]
        .rearrange("t b f -> (t b f)")
        .bitcast(mybir.dt.float32)[0:128]
        .rearrange("(p o) -> p o", o=1)
    )

    FREE = _B * _FS  # 4096 elements per slot per partition

    with tile.TileContext(nc) as tc:
        with (
            tc.tile_pool(name="xp8", bufs=1) as xp8,
            tc.tile_pool(name="xp16", bufs=1) as xp16,
            tc.tile_pool(name="twp", bufs=1) as twp,
            tc.tile_pool(name="stp", bufs=1) as stp,
            tc.tile_pool(name="y8p", bufs=1) as y8p,
            tc.tile_pool(name="scp", bufs=1) as scp,
            tc.tile_pool(name="psp", bufs=6, space="PSUM") as psp,
        ):
            # w transposed to [c, f], then banded Toeplitz lhsT: for each
            # output column m, T[m+c, f, m] = w'[f, c] — a partition-offset
            # copy of the transposed weight placed diagonally.
            wt = twp.tile([32, _FS], mybir.dt.float16, tag="wt")
            nc.sync.dma_start(out=wt[0 : _C + 1, :], in_=w_d.rearrange("f c -> c f"))
            tw = twp.tile([128, _FS * _ST], mybir.dt.float16, tag="tw")
            nc.gpsimd.memset(tw[:, :], 0.0)
            twv = tw[:].rearrange("p (f m) -> p f m", f=_FS, m=_ST)
            for m in range(_ST):
                nc.sync.dma_start(out=twv[m : m + _C + 1, :, m], in_=wt[0 : _C + 1, :])

            sc = scp.tile([128, 1], mybir.dt.float32, tag="sc")
            inv = scp.tile([128, 1], mybir.dt.float32, tag="inv")

            xt8 = xp8.tile([128, nsl * FREE], mybir.dt.uint8, tag="x8")
            for s in range(nsl):
                nc.sync.dma_start(
                    out=xt8[:, s * FREE : (s + 1) * FREE],
                    in_=x_d[s * _ST : s * _ST + 128, :, :].rearrange("t b f -> t (b f)"),
                )
            # x arrives biased: u8 = round(x/sx) + 128; undo the bias in the
            # same instruction that upcasts to fp16.
            xt16 = xp16.tile([128, nsl * FREE], mybir.dt.float16, tag="x16")
            nc.vector.tensor_scalar_add(xt16[:, :], xt8[:, :], -128.0)
            xrv = xt16[:].rearrange("p (s b f) -> p s b f", s=nsl, b=_B, f=_FS)

            st = stp.tile([128, nsl * FREE], mybir.dt.float16, tag="st")
            stv = st[:].rearrange("p (s b f) -> p f s b", s=nsl, b=_B, f=_FS)

            nfree = nsl * _B
            for fp in range(_FS // 2):
                ps = psp.tile([128, 2 * nfree], mybir.dt.float32, tag="ps")
                for fh in range(2):
                    f = 2 * fp + fh
                    nc.tensor.matmul(
                        ps[0:_ST, fh * nfree : (fh + 1) * nfree],
                        twv[:, f, :],
                        xrv[:, :, :, f],
                        start=True,
                        stop=True,
                    )
                pv = ps[:].rearrange("p (f s b) -> p f s b", f=2, s=nsl, b=_B)
                nc.vector.tensor_copy(
                    stv[0:_ST, 2 * fp : 2 * fp + 2, :, :], pv[0:_ST, :, :, :]
                )

            # exact per-partition (= per output row mod ST) absmax, then
            # y8 = y * (QCAP/absmax) + 128.5 stored as uint8; the exact
            # multiplier inv is downloaded so the host can invert it.
            nc.vector.tensor_reduce(
                sc[:, 0:1],
                st[:, :],
                mybir.AxisListType.X,
                mybir.AluOpType.max,
                apply_absolute_value=True,
            )
            nc.vector.reciprocal(inv[:, 0:1], sc[:, 0:1])
            nc.vector.tensor_scalar_mul(inv[:, 0:1], inv[:, 0:1], _QCAP)
            y8 = y8p.tile([128, nsl * FREE], mybir.dt.uint8, tag="y8")
            nc.vector.tensor_scalar(
                y8[:, :],
                st[:, :],
                inv[:, 0:1],
                128.5,
                mybir.AluOpType.mult,
                mybir.AluOpType.add,
            )
            sv = y8[:].rearrange("p (s b f) -> p s b f", s=nsl, b=_B, f=_FS)
            for s in range(nsl):
                nc.scalar.dma_start(
                    out=y_d[s * _ST : (s + 1) * _ST, :, :].rearrange("t b f -> t (b f)"),
                    in_=sv[0:_ST, s, :, :],
                )
            nc.scalar.dma_start(out=s_d, in_=inv[:, :])
    nc.compile()
    return nc


def _make_dispatch(nsl):
    import jax
    from jax.sharding import Mesh, NamedSharding, PartitionSpec

    try:
        from jax.experimental.shard_map import shard_map
    except ImportError:  # newer jax
        from jax import shard_map  # type: ignore

    from concourse import bass2jax, mybir

    nc = _build(nsl)
    bass2jax.install_neuronx_cc_hook()

    partition_name = nc.partition_id_tensor.name if nc.partition_id_tensor else None
    in_names, out_names, out_avals = [], [], []
    for alloc in nc.m.functions[0].allocations:
        if not isinstance(alloc, mybir.MemoryLocationSet):
            continue
        if alloc.kind == "ExternalInput":
            if alloc.memorylocations[0].name != partition_name:
                in_names.append(alloc.memorylocations[0].name)
        elif alloc.kind == "ExternalOutput":
            out_names.append(alloc.memorylocations[0].name)
            out_avals.append(
                jax.core.ShapedArray(tuple(alloc.tensor_shape), mybir.dt.np(alloc.dtype))
            )

    all_in_names = list(in_names)
    if partition_name is not None:
        all_in_names.append(partition_name)

    def _body(*args):
        operands = list(args)
        if partition_name is not None:
            operands.append(bass2jax.partition_id_tensor())
        outs = bass2jax._bass_exec_p.bind(
            *operands,
            out_avals=tuple(out_avals),
            in_names=tuple(all_in_names),
            out_names=tuple(out_names),
            lowering_input_output_aliases=(),
            sim_require_finite=True,
            sim_require_nnan=True,
            nc=nc,
        )
        return tuple(outs)

    devices = jax.devices()[:_NC]
    mesh = Mesh(np.asarray(devices), ("core",))
    sharding = NamedSharding(mesh, PartitionSpec("core"))
    fn = shard_map(
        _body,
        mesh=mesh,
        in_specs=(PartitionSpec("core"),) * len(in_names),
        out_specs=(PartitionSpec("core"),) * len(out_names),
        check_rep=False,
    )
    shape_map = {
        "xs": ((_NC * (_sin(nsl) + _WROWS), _B, _FS), np.uint8),
    }
    example = [
        jax.ShapeDtypeStruct(*shape_map[n], sharding=sharding) for n in in_names
    ]
    compiled = bass2jax.fast_dispatch_compile(
        lambda: jax.jit(fn).lower(*example).compile()
    )
    return compiled, in_names, out_names, sharding


def _get_ctx():
    global _ctx
    with _ctx_lock:
        if _ctx is None:
            progs = {nsl: _make_dispatch(nsl) for nsl in sorted({c[1] for c in _CHUNKS})}
            # persistent staging; biased-zero padding rows stay 128 forever
            xq = [
                np.full((_NC * (_sin(nsl) + _WROWS), _B, _FS), 128, np.uint8)
                for _, nsl in _CHUNKS
            ]
            y = np.zeros((_S, _B, _F), np.float32)  # touch pages once here
            _ctx = (progs, xq, y)
    return _ctx


def _quantize_chunk(x, weight, k, xq_k):
    """Quantize chunk k of x to biased uint8 into the persistent buffer
    xq_k and return the per-core folded fp16 weights.

    u8 = trunc(x/sx + 128.5) = round_half_up(x/sx) + 128, in [1, 255] by
    construction (|x/sx| <= 127), so no rint/clip passes are needed.
    """
    base, nsl = _CHUNKS[k]
    SIN = _sin(nsl)
    STRIDE = SIN + _WROWS
    real = min(SIN, _S - base)
    xs = x[base : base + real]
    amax = np.maximum(xs.max(axis=(0, 1)), -xs.min(axis=(0, 1)))
    amax = np.maximum(amax, 1e-20).astype(np.float32)
    inv_sx = (127.0 / amax).astype(np.float32)
    w_all = (weight * (amax / 127.0)[:, None]).astype(np.float16)  # (F, 21)
    for c in range(_NC):
        fs = c * _FS
        v = xs[:, :, fs : fs + _FS] * inv_sx[fs : fs + _FS]
        v += 128.5
        xq_k[c * STRIDE : c * STRIDE + real] = v.astype(np.uint8)
        # rows past the end stay biased-zero (128) from init; the sideband
        # rows carry this core's folded weights as raw fp16 bytes
        side = xq_k[c * STRIDE + SIN : c * STRIDE + SIN + _WROWS].reshape(-1)
        wb = w_all[fs : fs + _FS].tobytes()
        side[: len(wb)] = np.frombuffer(wb, np.uint8)


def _run_chunk(k, xq_k, progs, y_out, up_seq, down_seq):
    """Upload, execute, download, dequantize one chunk (runs in a pool
    thread; the transfers release the GIL so chunks overlap)."""
    import jax

    base, nsl = _CHUNKS[k]
    SOUT = _sout(nsl)
    OSTRIDE = SOUT + 1
    compiled, in_names, out_names, sharding = progs[nsl]
    up_seq.wait(k)
    try:
        arg = jax.device_put(xq_k, sharding)
        arg.block_until_ready()
    finally:
        up_seq.advance()
    outs = compiled(arg)
    # wait for the execute BEFORE claiming the download turn, so the
    # exclusive down-pipe turn is only held while bytes actually move
    outs[0].block_until_ready()
    down_seq.wait(k)
    try:
        y_u8 = np.asarray(outs[0])  # (NC*(SOUT+1), B, FS) uint8
    finally:
        down_seq.advance()

    rows = min(SOUT, _S - base)
    pp = np.arange(rows) % _ST
    tmp = np.empty((rows, _B, _FS), np.float32)
    for c in range(_NC):
        inv_dl = y_u8[c * OSTRIDE + SOUT].reshape(-1)[:512].view(np.float32)
        s_row = np.float32(1.0) / inv_dl[pp]
        # single-pass cast+unbias, then scale directly into the strided view
        np.subtract(
            y_u8[c * OSTRIDE : c * OSTRIDE + rows], np.float32(128.0),
            out=tmp, dtype=np.float32,
        )
        np.multiply(
            tmp, s_row[:, None, None],
            out=y_out[base : base + rows, :, c * _FS : (c + 1) * _FS],
        )


def kernel(x: np.ndarray, weight: np.ndarray) -> np.ndarray:
    progs, xq_bufs, y_out = _get_ctx()

    x = np.asarray(x)
    weight = np.asarray(weight)

    up_seq, down_seq = _Sequencer(), _Sequencer()
    futs = []
    for k in range(len(_CHUNKS)):
        _quantize_chunk(x, weight, k, xq_bufs[k])
        futs.append(
            _NET.submit(_run_chunk, k, xq_bufs[k], progs, y_out, up_seq, down_seq)
        )
    for f in futs:
        f.result()
    return y_out
